# revision 24
# baseline (speedup 1.0000x reference)
"""Trainium2 Bass kernel for the MHSA bottleneck block (v6).

Contract: kernel(**inputs) takes the FULL unsharded inputs (as produced by
setup_inputs()) and returns the FULL [64, 2048, 14, 14] float32 output.
Internally shards data-parallel over batch: 8 images per NeuronCore, 8 cores.

Precision plan (validated against the reference in fp64 simulation and on HW,
rel_err ~1.43e-2 vs the 2e-2 gate):
  - conv1, q/k/v projections, attn*v and conv3 run in fp8e4 (e4m3) with
    DoubleRow perf mode (2 contraction planes per instruction, ~2x PE rate).
  - The attention logits (q^T k + pos^T q) and softmax run in bf16/fp32
    (logits reach ~46 and feed exp(), fp8 there would blow up the softmax).
  - All fp8 scales are powers of 2, folded into adjacent activation
    scales/biases so no extra ops are spent on rescaling.
  - The residual + bn3 bias ride the conv3 psum via an S3*I identity matmul
    against host-precomputed xr = bf16(x + t3).

Layouts are pair-major and flat per partition so every DMA is one
contiguous run per partition (fast descriptor generation, full HBM bursts).
"""
import sys

sys.path.insert(0, '/opt/trn_rl_repo')

import numpy as np
import ml_dtypes

# Problem constants (hardcoded per the harness contract).
B, CIN, P, H, W = 64, 2048, 512, 14, 14
EPS = 1e-5
N = H * W            # 196 pixels
NCORES = 8
BPC = B // NCORES    # 8 images per core
NPAIR = BPC // 2     # 4 image pairs per core
KC1 = CIN // 128     # 16 input-channel chunks
KP1 = KC1 // 2       # 8 chunk-pairs for fp8 DoubleRow
PC = P // 128        # 4 chunks of the 512-dim
N2 = 2 * N           # 392 = free dim for image-pair matmuls

# n/m chunking of the 196-pixel dim: 128 + 68
NCHUNKS = [(0, 128), (128, 68)]
# fp8 DoubleRow k-plane strides must be 16-byte aligned -> padded free dims
N2P = 400            # N2=392 padded
NP = 208             # N=196 padded

# fp8 power-of-2 scales (>=2.4x margin to fp8e4 max 240: x absmax 5.4,
# h1 max 6.1, q/k absmax 1.6, vT absmax 1.4, attn <= 1, h2 max 1.7).
SX8 = 16.0
SH1 = 1.0            # h1 stored unscaled in fp8
SH2 = 32.0
SV = 64.0            # = lamv * SH1, so the vT psum->fp8 copy needs no scale
SA = 128.0           # folded into the softmax-normalize multiply

F8 = ml_dtypes.float8_e4m3
BF = ml_dtypes.bfloat16

_CACHE = {}


def _pow2scale(a, target=32.0):
    m = float(np.abs(a).max())
    return float(2.0 ** np.floor(np.log2(target / m)))


def _build(lam1, lamv, lam3, lamqk):
    import concourse.bass as bass  # noqa: F401
    import concourse.mybir as mybir
    import concourse.tile as tile
    from concourse import bacc
    from concourse.masks import make_identity

    f32 = mybir.dt.float32
    bf16 = mybir.dt.bfloat16
    f8 = mybir.dt.float8e4
    DR = mybir.MatmulPerfMode.DoubleRow

    c1s = 1.0 / (lam1 * SX8)          # conv1 psum -> true scale
    assert lamv * SH1 == SV           # vT copy must be scale-free
    s3inv = 1.0 / (lam3 * SH2)        # conv3 psum -> true scale
    s3 = lam3 * SH2                   # residual identity matmul scale
    qks = 1.0 / (lamqk * SH1)         # q/k psum -> true scale

    nc = bacc.Bacc(None, target_bir_lowering=False, debug=False)

    x8_d = nc.declare_dram_parameter("x8", [128, NPAIR, KP1, 2, N2P], f8,
                                     isOutput=False)
    xr_d = nc.declare_dram_parameter("xr", [128, NPAIR, KC1 * N2], bf16,
                                     isOutput=False)
    w1_d = nc.declare_dram_parameter("w1q", [128, KP1, 2, P], f8,
                                     isOutput=False)
    wqk_d = nc.declare_dram_parameter("wqk", [128, 2, 2, 2 * P], f8,
                                      isOutput=False)
    wv_d = nc.declare_dram_parameter("wvq", [128, 2, 2, P], f8,
                                     isOutput=False)
    w3_d = nc.declare_dram_parameter("w3q", [128, 2, 2, CIN], f8,
                                     isOutput=False)
    pos_d = nc.declare_dram_parameter("pos", [128, PC, N], bf16,
                                      isOutput=False)
    tb_d = nc.declare_dram_parameter("tb", [128, 3 * PC], f32,
                                     isOutput=False)
    y_d = nc.declare_dram_parameter("y", [NPAIR, 128, KC1 * N2], bf16,
                                    isOutput=True)

    with tile.TileContext(nc) as tc:
        with (
            tc.tile_pool(name="const", bufs=1) as const,
            tc.tile_pool(name="work", bufs=1) as work,
            tc.tile_pool(name="ps_mm", bufs=4, space="PSUM") as ps_mm,
            tc.tile_pool(name="ps_att", bufs=2, space="PSUM") as ps_att,
            tc.tile_pool(name="ps_tr", bufs=2, space="PSUM") as ps_tr,
        ):
            # ---- inputs / weights (loaded once, fully resident) ----
            w1q = const.tile([128, KP1, 2, P], f8)
            wqk = const.tile([128, 2, 2, 2 * P], f8)
            wvq = const.tile([128, 2, 2, P], f8)
            w3q = const.tile([128, 2, 2, CIN], f8)
            pos = const.tile([128, PC, N], bf16)
            tb = const.tile([128, 3 * PC], f32)
            xr = const.tile([128, NPAIR, KC1 * N2], bf16)
            ident = const.tile([128, 128], bf16)
            x8 = work.tile([128, NPAIR, KP1, 2, N2P], f8, name="x8")
            ypair = [work.tile([128, KC1, N2], bf16, name=f"yp_{i}",
                               tag=f"yp_{i}") for i in range(2)]

            # startup-critical DMAs first, split across queues; conv1 is
            # kp-outer so compute starts as soon as the first pieces land
            for kq in range(0, KP1, 2):
                nc.sync.dma_start(out=x8[:, 0, kq:kq + 2],
                                  in_=x8_d[:, 0, kq:kq + 2])
                nc.sync.dma_start(out=w1q[:, kq:kq + 2],
                                  in_=w1_d[:, kq:kq + 2, :, :])
            nc.sync.dma_start(out=wqk, in_=wqk_d[:, :, :, :])
            nc.sync.dma_start(out=wvq, in_=wv_d[:, :, :, :])
            for p in range(1, NPAIR):
                nc.sync.dma_start(out=x8[:, p, 0:4], in_=x8_d[:, p, 0:4])
                nc.sync.dma_start(out=x8[:, p, 4:8], in_=x8_d[:, p, 4:8])
            nc.sync.dma_start(out=tb, in_=tb_d[:, :])
            nc.sync.dma_start(out=pos, in_=pos_d[:, :, :])
            nc.sync.dma_start(out=w3q, in_=w3_d[:, :, :, :])
            make_identity(nc, ident)
            scr = const.tile([1, 1], f32)
            ids3 = const.tile([128, 128], bf16)
            nc.gpsimd.memset(ids3, 0.0)
            nc.gpsimd.affine_select(
                out=ids3, in_=ids3,
                compare_op=mybir.AluOpType.not_equal, fill=s3,
                base=0, pattern=[[-1, 128]], channel_multiplier=1)
            # warm the scalar engine's Exp table during the idle startup
            nc.scalar.activation(scr, ident[0:1, 0:1],
                                 mybir.ActivationFunctionType.Exp)

            t1 = tb[:, 0:PC]
            s2h = tb[:, PC:2 * PC]
            t2h = tb[:, 2 * PC:3 * PC]

            Exp = mybir.ActivationFunctionType.Exp
            Relu = mybir.ActivationFunctionType.Relu
            Copy = mybir.ActivationFunctionType.Copy
            Mult = mybir.AluOpType.mult
            Maxi = mybir.AluOpType.max

            # ---- per-pair tiles (all pairs resident) ----
            def wt(shape, dtype, nm):
                return work.tile(shape, dtype, name=nm, tag=nm)

            h1q = [wt([128, 2, 2, N2P], f8, f"h1q_{p}") for p in range(NPAIR)]
            qsb = [wt([128, PC, N2], bf16, f"q_{p}") for p in range(NPAIR)]
            ksb = [wt([128, PC, N2], bf16, f"k_{p}") for p in range(NPAIR)]
            vT = [wt([128, 2, 2, P], f8, f"vT_{p}") for p in range(NPAIR)]
            attnT = [wt([128, 2, 2, NP], f8, f"aT_{p}") for p in range(NPAIR)]
            praw = [wt([128, 2, 2, N], bf16, f"pr_{p}") for p in range(NPAIR)]
            pnrm = [wt([128, 2, 2, N], bf16, f"pn_{p}") for p in range(NPAIR)]
            ssum = [wt([128, 2, 2, 1], f32, f"ss_{p}") for p in range(NPAIR)]
            rsum = [wt([128, 2, 2, 1], f32, f"rs_{p}") for p in range(NPAIR)]
            h2q = [wt([128, 2, 2, N2P], f8, f"h2_{p}") for p in range(NPAIR)]

            # ---- phase 1: conv1 + bn1 + relu (fp8 DoubleRow) ----
            # kp-outer with 4 concurrent psum groups: the first matmul only
            # needs the first x8/w1q pieces, so compute overlaps the load
            def emit_conv1(p):
                cps4 = [ps_mm.tile([128, 512], f32, name=f"cps{oc}",
                                   tag="mm") for oc in range(PC)]
                for kp in range(KP1):
                    for oc in range(PC):
                        nc.tensor.matmul(
                            cps4[oc][:, :N2],
                            w1q[:, kp, :, oc * 128:(oc + 1) * 128],
                            x8[:, p, kp, :, 0:N2],
                            start=(kp == 0), stop=(kp == KP1 - 1),
                            perf_mode=DR,
                        )
                for oc in range(PC):
                    nc.scalar.activation(h1q[p][:, oc // 2, oc % 2, :N2],
                                         cps4[oc][:, :N2], Relu,
                                         bias=t1[:, oc:oc + 1], scale=c1s)
                # gate pair p's residual load on conv1(p)'s first output so
                # it does not steal HBM bandwidth from the startup loads
                nc.vector.tensor_copy(xr[:, p, 0:1], h1q[p][:, 0, 0, 0:1])
                nc.sync.dma_start(out=xr[:, p], in_=xr_d[:, p])

            # ---- phase 2: q/k projection (fp8 DoubleRow) ----
            def emit_qk(p):
                for oc in range(2 * PC):
                    qps = ps_mm.tile([128, 512], f32, name="qps", tag="mm")
                    for kp in range(2):
                        nc.tensor.matmul(
                            qps[:, :N2],
                            wqk[:, kp, :, oc * 128:(oc + 1) * 128],
                            h1q[p][:, kp, :, :N2],
                            start=(kp == 0), stop=(kp == 1),
                            perf_mode=DR,
                        )
                    dst = qsb[p] if oc < PC else ksb[p]
                    c4 = oc % PC
                    if oc % 2 == 0:
                        nc.vector.tensor_scalar_mul(dst[:, c4, :],
                                                    qps[:, :N2], qks)
                    else:
                        nc.scalar.activation(dst[:, c4, :], qps[:, :N2], Copy,
                                             scale=qks)

            # ---- phase 3: v projection -> vT (fp8 DoubleRow) ----
            def emit_v(p):
                for j in range(2):
                    # zero fp8 pads (partition rows 68..127 of the second
                    # m-chunk) so the DoubleRow attn*v matmul sees no garbage;
                    # start at 64 (quadrant-aligned) — rows 64..67 are
                    # overwritten by the real copies emitted later
                    nc.vector.memset(vT[p][64:128, j, 1, :], 0.0)
                    nc.vector.memset(attnT[p][64:128, j, 1, :], 0.0)
                    for mi, (m0, msz) in enumerate(NCHUNKS):
                        vps = ps_mm.tile([128, 512], f32, name="vps", tag="mm")
                        for kp in range(2):
                            nc.tensor.matmul(
                                vps[:msz, :],
                                h1q[p][:, kp, :, j * N + m0:j * N + m0 + msz],
                                wvq[:, kp, :, :],
                                start=(kp == 0), stop=(kp == 1),
                                perf_mode=DR,
                            )
                        if j == 0:
                            nc.scalar.activation(vT[p][:msz, j, mi, :],
                                                 vps[:msz, :], Copy)
                        else:
                            nc.vector.tensor_copy(vT[p][:msz, j, mi, :],
                                                  vps[:msz, :])

            # ---- phase 4: attention (logits bf16, softmax, attn*v fp8) ----
            def emit_logits_softmax(p):
                for j in range(2):
                    for ni, (n0, nsz) in enumerate(NCHUNKS):
                        lps = ps_att.tile([128, N], f32, name="lps",
                                          tag="att")
                        for pc in range(PC):
                            nc.tensor.matmul(
                                lps[:nsz, :],
                                qsb[p][:, pc, j * N + n0:j * N + n0 + nsz],
                                ksb[p][:, pc, j * N:(j + 1) * N],
                                start=(pc == 0), stop=False,
                            )
                        for pc in range(PC):
                            nc.tensor.matmul(
                                lps[:nsz, :],
                                pos[:, pc, n0:n0 + nsz],
                                qsb[p][:, pc, j * N:(j + 1) * N],
                                start=False, stop=(pc == PC - 1),
                            )
                        # softmax over free dim (logits <= ~46: exp stays
                        # finite in fp32, no max-subtraction needed)
                        nc.scalar.activation(praw[p][:nsz, j, ni, :],
                                             lps[:nsz, :], Exp,
                                             accum_out=ssum[p][:nsz, j, ni, :])
                        nc.vector.reciprocal(rsum[p][:nsz, j, ni, :],
                                             ssum[p][:nsz, j, ni, :])
                        # normalize and pre-scale by SA for the fp8 attnT
                        nc.vector.tensor_scalar(pnrm[p][:nsz, j, ni, :],
                                                praw[p][:nsz, j, ni, :],
                                                rsum[p][:nsz, j, ni, :], SA,
                                                op0=Mult, op1=Mult)

            def emit_tr_av(p):
                for j in range(2):
                    for ni, (n0, nsz) in enumerate(NCHUNKS):
                        for mi, (m0, msz) in enumerate(NCHUNKS):
                            tps = ps_tr.tile([128, 128], bf16, name="tps",
                                             tag="tr")
                            nc.tensor.transpose(
                                tps[:msz, :nsz],
                                pnrm[p][:nsz, j, ni, m0:m0 + msz],
                                ident[:nsz, :nsz])
                            if j == 0:
                                nc.scalar.activation(
                                    attnT[p][:msz, j, mi, n0:n0 + nsz],
                                    tps[:msz, :nsz], Copy)
                            else:
                                nc.vector.tensor_copy(
                                    attnT[p][:msz, j, mi, n0:n0 + nsz],
                                    tps[:msz, :nsz])
                for j in range(2):
                    for c4 in range(PC):
                        aps = ps_att.tile([128, N], f32, name="aps",
                                          tag="att")
                        nc.tensor.matmul(
                            aps[:, :],
                            vT[p][:, j, :, c4 * 128:(c4 + 1) * 128],
                            attnT[p][:, j, :, :N],
                            start=True, stop=True,
                            perf_mode=DR,
                        )
                        nc.scalar.activation(
                            h2q[p][:, c4 // 2, c4 % 2, j * N:(j + 1) * N],
                            aps[:, :], Relu,
                            bias=t2h[:, c4:c4 + 1], scale=s2h[:, c4:c4 + 1])

            # ---- phase 5: conv3 + bn3 + residual + relu (fp8 DoubleRow) ----
            def emit_conv3(p):
                yp = ypair[p % 2]
                for oc in range(KC1):
                    ops = ps_mm.tile([128, 512], f32, name="ops", tag="mm")
                    for kp in range(2):
                        nc.tensor.matmul(
                            ops[:, :N2],
                            w3q[:, kp, :, oc * 128:(oc + 1) * 128],
                            h2q[p][:, kp, :, :N2],
                            start=(kp == 0), stop=False,
                            perf_mode=DR,
                        )
                    # psum += S3 * xr   (residual + bn3 bias via the PE)
                    nc.tensor.matmul(
                        ops[:, :N2], ids3,
                        xr[:, p, oc * N2:(oc + 1) * N2],
                        start=False, stop=True)
                    if oc % 2 == 0 and p > 0:
                        nc.scalar.activation(yp[:, oc, :], ops[:, :N2], Relu,
                                             scale=s3inv)
                    else:
                        nc.vector.tensor_scalar(yp[:, oc, :], ops[:, :N2],
                                                s3inv, 0.0,
                                                op0=Mult, op1=Maxi)
                    # stream out in halves to shrink the tail DMA
                    if oc % 8 == 7:
                        q0 = oc - 7
                        nc.sync.dma_start(
                            out=y_d[p, :, q0 * N2:(oc + 1) * N2],
                            in_=yp[:, q0:oc + 1, :])

            # ---- emission: front phases interleaved (qk/v of pair p
            # cover pair p+1's input transfer); attention and conv3
            # phase-major ----
            for p in range(NPAIR):
                emit_conv1(p)
                emit_qk(p)
                emit_v(p)
            for p in range(NPAIR):
                emit_logits_softmax(p)
                if p >= 1:
                    emit_tr_av(p - 1)
            emit_tr_av(NPAIR - 1)
            for p in range(NPAIR):
                emit_conv3(p)

    nc.compile()
    return nc


def _prep_inputs(x, w1, g1, b1, m1, v1, wqkv, rel_h, rel_w,
                 g2, b2, m2, v2, w3, g3, b3, m3, v3):
    f = np.float32
    x = np.asarray(x, f)
    s1 = (g1 / np.sqrt(v1 + EPS)).astype(f)
    t1 = (b1 - m1 * s1).astype(f)
    s2 = (g2 / np.sqrt(v2 + EPS)).astype(f)
    t2 = (b2 - m2 * s2).astype(f)
    s3 = (g3 / np.sqrt(v3 + EPS)).astype(f)
    t3 = (b3 - m3 * s3).astype(f)

    w1p = (w1 * s1[:, None]).astype(f)                    # [512, 2048]
    wqk = wqkv[:2 * P].astype(f)                          # [1024, 512]
    wv = wqkv[2 * P:].astype(f)                           # [512, 512]
    w3p = (w3 * s3[:, None]).astype(f)                    # [2048, 512]

    lam1 = _pow2scale(w1p)
    lamv = SV / SH1   # fixed so the on-chip vT copy needs no rescale
    assert float(np.abs(wv).max()) * lamv < 200.0
    lam3 = _pow2scale(w3p)
    lamqk = _pow2scale(wqk)

    # fp8 weights in DoubleRow layouts
    w1q = np.ascontiguousarray(
        (w1p.T * lam1).reshape(KP1, 2, 128, P).transpose(2, 0, 1, 3)
    ).astype(F8)                                          # [128, 8, 2, 512]
    wqkt = np.ascontiguousarray(
        (wqk.T * lamqk).reshape(2, 2, 128, 2 * P).transpose(2, 0, 1, 3)
    ).astype(F8)                                          # [128, 2, 2, 1024]
    wvq = np.ascontiguousarray(
        (wv.T * lamv).reshape(2, 2, 128, P).transpose(2, 0, 1, 3)
    ).astype(F8)                                          # [128, 2, 2, 512]
    w3q = np.ascontiguousarray(
        (w3p.T * lam3).reshape(2, 2, 128, CIN).transpose(2, 0, 1, 3)
    ).astype(F8)                                          # [128, 2, 2, 2048]
    posh = np.ascontiguousarray(
        (rel_h + rel_w).reshape(P, N).astype(f)
        .reshape(PC, 128, N).transpose(1, 0, 2)).astype(BF)

    tb = np.empty((128, 3 * PC), f)
    tb[:, 0:PC] = t1.reshape(PC, 128).T
    tb[:, PC:2 * PC] = (s2 * (SH2 / (SV * SA))).reshape(PC, 128).T
    tb[:, 2 * PC:3 * PC] = (t2 * SH2).reshape(PC, 128).T

    shared = dict(w1q=w1q, wqk=wqkt, wvq=wvq, w3q=w3q, pos=posh, tb=tb)

    in_maps = []
    for c in range(NCORES):
        xc = x[c * BPC:(c + 1) * BPC].reshape(BPC, CIN, N)
        # fp8 conv1 input, pair-major padded: [128, pair, kp, kin, 400]
        a = (xc * SX8).reshape(NPAIR, 2, KP1, 2, 128, N) \
            .transpose(4, 0, 2, 3, 1, 5).reshape(128, NPAIR, KP1, 2, N2)
        x8 = np.zeros((128, NPAIR, KP1, 2, N2P), F8)
        x8[..., :N2] = a.astype(F8)
        # bf16 residual (+ bn3 bias folded), pair-major flat
        xr = np.ascontiguousarray(
            (xc + t3[None, :, None]).reshape(NPAIR, 2, KC1, 128, N)
            .transpose(3, 0, 2, 1, 4).reshape(128, NPAIR, KC1 * N2)
        ).astype(BF)
        in_maps.append(dict(shared, x8=x8, xr=xr))
    return in_maps, (lam1, lamv, lam3, lamqk)


def _run(in_maps, lams, trace=False):
    from concourse.bass_utils import run_bass_kernel_spmd
    key = ("nc", lams)
    if key not in _CACHE:
        _CACHE[key] = _build(*lams)
    nc = _CACHE[key]
    return run_bass_kernel_spmd(nc, in_maps, core_ids=list(range(NCORES)),
                                trace=trace)


def _unpack(yc):
    """[4, 128, KC1*N2] bf16 -> [8, CIN, H, W] f32 for one core."""
    y = np.asarray(yc).astype(np.float32).reshape(NPAIR, 128, KC1, 2, N)
    return y.transpose(0, 3, 2, 1, 4).reshape(BPC, CIN, H, W)


def kernel(**inputs):
    in_maps, lams = _prep_inputs(**inputs)
    res = _run(in_maps, lams)
    out = np.empty((B, CIN, H, W), np.float32)
    for c in range(NCORES):
        out[c * BPC:(c + 1) * BPC] = _unpack(res.results[c]["y"])
    return out
